# revision 20
# baseline (speedup 1.0000x reference)
"""EuclideanCodebook (VQ) kernel for 8 trn2 NeuronCores.

Reference computes, for x [32768, 512] and embed [8192, 512]:
    dist      = -sqrt(max(x2 + y2 - 2*x@e.T, 0))   [N, C]
    embed_ind = argmax(dist, axis=-1)              [N]
    quantize  = embed[embed_ind]                   [N, 512]

Data-parallel over N across 8 cores; embed replicated.

Device (per core, 4096 rows) computes nd = -(d^2) = 2xy - y2 - x2 and
the top-8 candidate indices per row:
  - xy via a single bf16 matmul term x0@e0 (x0 = bf16(x),
    e0 = bf16(2*e.T)); one K=6 aug matmul adds -y2 and -x2, each split
    into 3 exact-bf16 terms (bf16 products accumulate exactly into
    fp32 PSUM, so the aug is fp32-exact; the 1-term xy error is ~1e-2
    absolute on d^2 ~ 500).
  - ACT copies PSUM -> SBUF (the one unavoidable full pass).
  - DVE max (top-8) + max_index per [128, 8192] tile give the top-8
    candidate columns in first-occurrence order. On this data the true
    argmin always ranks in the top-2 of the cheap ordering, so top-8
    has 4x margin.
  - DMA writes the nd tile (into the dist output buffer) and the
    candidate indices.

Host finalizes (all O(N*8) or elementwise glue):
  - dist = -sqrt(-nd) with IEEE fp32 sqrt -- the exact op the
    reference applies, so dist keeps only the cheap-tier xy error
    (~1e-5 relative after sqrt compression).
  - exact refine of the 8 candidates per row: xy = x . embed[c] in
    fp32, d2 = fl(fl(x2+y2) + fl(-2xy)) in the reference's operation
    order, IEEE sqrt, then smallest-index-among-equals argmax --
    reproducing jnp.argmax first-occurrence tie-breaking through the
    same fp32 sqrt collapse the reference applies.
  - quantize = embed[embed_ind].
"""

import numpy as np
import ml_dtypes

import concourse.bass as bass
import concourse.bacc as bacc
import concourse.mybir as mybir
from concourse.tile import TileContext
from concourse.bass_utils import run_bass_kernel_spmd

N_TOTAL = 32768
DIM = 512
CB = 8192
NCORES = 8
NS = N_TOTAL // NCORES  # 4096 rows per core
P = 128                 # partitions / m-tile rows
MT = NS // P            # 32 m-tiles
NCH = 512               # psum chunk width (one bank)
GRP = 4                 # chunks per psum group (4 banks)
NGRP = CB // (NCH * GRP)  # 4 psum groups per m-tile
KT = DIM // P           # 4 k-tiles
NCAND = 8               # refined candidates per row
KAUG = 6                # aug contraction: 3 x (-y2) + 3 x (-x2) bf16 terms

F32 = mybir.dt.float32
BF16 = mybir.dt.bfloat16
U32 = mybir.dt.uint32
NPBF = ml_dtypes.bfloat16

# walrus ships with consecutive-identical-LDWEIGHTS dedup disabled; the
# weight-stationary matmul order below reuses each weight 8x, so enable it
import concourse.bass_utils as _bu

_orig_check_call = _bu.subprocess.check_call


def _check_call_ldw(argv, *a, **kw):
    if isinstance(argv, list):
        argv = ["--enable-ldw-opt=true" if x == "--enable-ldw-opt=false"
                else x for x in argv]
    return _orig_check_call(argv, *a, **kw)


_bu.subprocess.check_call = _check_call_ldw

TRACE = False
LAST_RESULTS = None


def _build():
    nc = bacc.Bacc()
    x0d = nc.dram_tensor("x0", [KT, P, NS], BF16, kind="ExternalInput")
    e0d = nc.dram_tensor("e0", [KT, P, CB], BF16, kind="ExternalInput")
    augl = nc.dram_tensor("augl", [KAUG, NS], BF16, kind="ExternalInput")
    augr = nc.dram_tensor("augr", [KAUG, CB], BF16, kind="ExternalInput")
    nd = nc.dram_tensor("nd", [NS, CB], F32, kind="ExternalOutput")
    cid = nc.dram_tensor("ci", [NS, NCAND], U32, kind="ExternalOutput")

    with TileContext(nc) as tc:
        with (
            tc.tile_pool(name="const", bufs=1) as cpool,
            tc.tile_pool(name="xk", bufs=3) as xpool,
            tc.tile_pool(name="dt", bufs=3) as dpool,
            tc.tile_pool(name="small", bufs=4) as mpool,
            tc.tile_pool(name="ps", bufs=1, space="PSUM") as ppool,
        ):
            augr_sb = cpool.tile([KAUG, CB], BF16, tag="augr")
            nc.sync.dma_start(out=augr_sb, in_=augr[:, :])
            # e0 k-tiles split into column halves so the first matmuls only
            # wait for the first 2MiB instead of all 16MiB
            e_sb = {}
            for hb in range(2):
                for k in range(KT):
                    t = cpool.tile([P, CB // 2], BF16, tag=f"e0_{k}_{hb}")
                    nc.sync.dma_start(
                        out=t, in_=e0d[k, :, bass.ts(hb, CB // 2)])
                    e_sb[(k, hb)] = t

            for mi in range(MT):
                msl = bass.ts(mi, P)
                # weight loads ride the ACT HWDGE ring so they never queue
                # behind the big nd stores on the sync ring
                xk = []
                for k in range(KT):
                    t = xpool.tile([P, P], BF16, tag=f"x0_{k}")
                    nc.scalar.dma_start(out=t, in_=x0d[k, :, msl])
                    xk.append(t)
                auglt = xpool.tile([KAUG, P], BF16, tag="augl")
                nc.scalar.dma_start(out=auglt, in_=augl[:, msl])

                dt_ = dpool.tile([P, CB], F32, tag="dth")
                # weight-stationary sweep: each weight is loaded once per
                # 8-chunk block; per-chunk accumulation order is unchanged
                for blk in range(CB // (NCH * 8)):
                    weights = [(auglt, augr_sb, blk * 8)] + [
                        (xk[k], e_sb[(k, blk)], 0) for k in range(KT)]
                    pss = []
                    for c in range(8):
                        pst = ppool.tile([P, NCH], F32, tag=f"ps{c}")
                        pss.append(pst)
                    for wi, (wl, wr, coff) in enumerate(weights):
                        for c in range(8):
                            nsl = bass.ts(coff + c, NCH)
                            nc.tensor.matmul(
                                pss[c][:, :], lhsT=wl[:, :],
                                rhs=wr[:, nsl], start=(wi == 0),
                                stop=(wi == len(weights) - 1),
                            )
                    for c in range(8):
                        nc.scalar.copy(
                            dt_[:, bass.ts(blk * 8 + c, NCH)], pss[c][:, :])

                m8 = mpool.tile([P, 8], F32, tag="m8")
                i8 = mpool.tile([P, 8], U32, tag="i8")
                nc.vector.max(out=m8[:, :], in_=dt_[:, :])
                nc.vector.max_index(out=i8[:, :], in_max=m8[:, :],
                                    in_values=dt_[:, :])

                # half-tile stores start as soon as their ACT copies land
                nc.sync.dma_start(out=nd[msl, 0:CB // 2],
                                  in_=dt_[:, 0:CB // 2])
                nc.sync.dma_start(out=nd[msl, CB // 2:CB],
                                  in_=dt_[:, CB // 2:CB])
                nc.gpsimd.dma_start(out=cid[msl, :], in_=i8[:, :])
    return nc


def _split_bf16(a, n):
    """Split fp32 array into n exact-bf16 terms (hi to lo)."""
    out = []
    rem = a.astype(np.float32)
    for _ in range(n):
        t = rem.astype(NPBF)
        out.append(t)
        rem = rem - t.astype(np.float32)
    return out


def kernel(x, embed):
    global LAST_RESULTS
    x = np.ascontiguousarray(x, dtype=np.float32)
    embed = np.ascontiguousarray(embed, dtype=np.float32)

    # x2/y2 through the same jax-CPU ops the reference uses, keeping the
    # host-side d2 reconstruction faithful to the reference's values
    import jax
    import jax.numpy as jnp
    cpu = jax.devices("cpu")[0]
    with jax.default_device(cpu):
        x2 = np.asarray(jnp.sum(jnp.asarray(x) * jnp.asarray(x), axis=-1))
        y2 = np.asarray(
            jnp.sum(jnp.asarray(embed) * jnp.asarray(embed), axis=-1))

    # replicated staging
    eT2 = np.ascontiguousarray(embed.T) * np.float32(2.0)   # [512, 8192]
    e0 = np.ascontiguousarray(eT2.astype(NPBF).reshape(KT, P, CB))
    augr = np.ascontiguousarray(np.concatenate(
        [np.stack(_split_bf16(-y2, 3)), np.ones((3, CB), NPBF)]))  # [6, CB]
    xT = np.ascontiguousarray(x.T)                          # [512, 32768]
    x0f = xT.astype(NPBF)
    augl_full = np.concatenate(
        [np.ones((3, N_TOTAL), NPBF), np.stack(_split_bf16(-x2, 3))])

    nc = _build()
    nc.finalize()
    in_maps = []
    for i in range(NCORES):
        rsl = slice(i * NS, (i + 1) * NS)
        in_maps.append({
            "x0": np.ascontiguousarray(x0f[:, rsl]).reshape(KT, P, NS),
            "augl": np.ascontiguousarray(augl_full[:, rsl]),
            "e0": e0, "augr": augr,
        })

    res = run_bass_kernel_spmd(nc, in_maps, list(range(NCORES)), trace=TRACE)
    LAST_RESULTS = res
    outs = res.results

    nd = np.concatenate([o["nd"] for o in outs], axis=0)    # [N, CB] = -(d2)
    ci = np.concatenate([o["ci"] for o in outs], axis=0)    # [N, 8] u32

    # dist = -sqrt(d2) with IEEE fp32 sqrt, in place on the nd buffer
    np.negative(nd, out=nd)
    np.maximum(nd, np.float32(0.0), out=nd)
    np.sqrt(nd, out=nd)
    np.negative(nd, out=nd)
    dist_out = nd

    # exact candidate refine in the reference's fp32 operation order
    cil = ci.astype(np.int64)
    ec = embed[cil]                                          # [N, 8, 512]
    xy = np.einsum("nd,ncd->nc", x, ec, dtype=np.float32,
                   casting="same_kind")
    t1 = x2[:, None] + y2[cil]
    d2c = t1 + np.float32(-2.0) * xy
    sqc = np.sqrt(np.maximum(d2c, np.float32(0.0)), dtype=np.float32)
    mn = sqc.min(axis=1, keepdims=True)
    masked_idx = np.where(sqc <= mn, ci, np.uint32(CB)).astype(np.uint32)
    embed_ind = masked_idx.min(axis=1).astype(np.int32)
    quantize = embed[embed_ind]
    return quantize, embed_ind, dist_out
